# revision 7
# baseline (speedup 1.0000x reference)
"""KAN encoder (2 KAN layers + relu + linear head) on 8 trn2 NeuronCores.

Strategy: data-parallel on batch (512 rows/core), all weights replicated,
feature-on-partition / batch-on-free layout throughout (no transposes).

The spline path is a dense matmul over (in*9) with B-spline bases computed
via the exact identity

    bases_k(x) * 6 = a^3 - 4*e^3,   a = relu(2 - s_k),  e = relu(a - 1),
    s_k = |2.5*x + 3.5 - k|

(the 1/6 and the spline_scaler are folded into the weights host-side; the
*4 is realized as (2e)^2 * e).

Performance notes vs the f32 baseline (720us):
  * weights, x and all feature tensors are fp16: halves HBM traffic and
    matmuls still run at 1 cycle/row (same as f32r).
  * the basis slices share relu scale/bias, so the relu/cube chain runs as
    WIDE [128, 4*512] ops: ACT does the 8 per-k Abs + silu + packed relus,
    DVE runs packed fp16 tensor_scalar (4x_2p) / tensor_tensor (2x_1p) ops.
    The baseline was vector-bound (DVE 93% busy, 533ns per f32
    tensor_tensor); here DVE sits at ~60%.
  * layer-0 features are computed once and kept in SBUF for both output
    halves (the baseline recomputed them).
  * features live in 3 tiles per chunk (silu slot + two 4-slot spline
    halves) and matmuls run j-major, so the tensor engine starts as soon
    as the first half-chain finishes instead of waiting for the full
    feature tile.
"""
import numpy as np
from contextlib import ExitStack

from concourse import bacc, tile, mybir
from concourse.bass_utils import run_bass_kernel_spmd

F32 = mybir.dt.float32
F16 = mybir.dt.float16
AF = mybir.ActivationFunctionType
ALU = mybir.AluOpType

B, D_IN, H0, H1, L = 4096, 1024, 2048, 1024, 512
NCORES = 8
BC = B // NCORES          # 512 batch cols per core
NB = 512                  # free-dim (batch) tile = full per-core batch
HW = 4 * NB               # half of the 8 spline slots

_cache = {}


def _build_full():
    nc = bacc.Bacc("TRN2", target_bir_lowering=False, debug=False,
                   num_devices=NCORES)

    x_d = nc.dram_tensor("x_d", [8, 128, BC], F16, kind="ExternalInput")
    w0_d = nc.dram_tensor("w0_d", [8, 2, 2, 128, 9 * NB], F16,
                          kind="ExternalInput")
    w1_d = nc.dram_tensor("w1_d", [16, 2, 128, 9 * NB], F16,
                          kind="ExternalInput")
    dw_d = nc.dram_tensor("dw_d", [8, 128, L], F16, kind="ExternalInput")
    db_d = nc.dram_tensor("db_d", [128, 4], F32, kind="ExternalInput")
    o_d = nc.dram_tensor("o_d", [4, 128, BC], F32, kind="ExternalOutput")

    with tile.TileContext(nc) as tc, ExitStack() as ctx:
        psum = ctx.enter_context(tc.tile_pool(name="psum", bufs=1,
                                              space="PSUM"))
        fpool = ctx.enter_context(tc.tile_pool(name="fpool", bufs=1))
        wpool = ctx.enter_context(tc.tile_pool(name="wpool", bufs=1))
        ipool = ctx.enter_context(tc.tile_pool(name="ipool", bufs=1))
        opool = ctx.enter_context(tc.tile_pool(name="opool", bufs=1))

        fics = [None] * 8     # current feats tiles (base, sp0, sp1) per chunk
        h0ts = [None] * 16    # layer-0 output chunks

        bias_tiles = {}

        def bias_ap(val):
            val = float(val)
            if val not in bias_tiles:
                t = opool.tile([128, 1], F32, tag=f"b{len(bias_tiles)}",
                               name=f"bias{len(bias_tiles)}")
                nc.gpsimd.memset(t[:, :], val)
                bias_tiles[val] = t
            return bias_tiles[val][:, :]

        def emit_feats(src_ap, tagp):
            """Returns (base, sp0, sp1): base (128,NB) = silu(src);
            spP (128,4*NB) slot q -> 6*bases_{4P+q}(src)."""
            fb = fpool.tile([128, NB], F16, tag=f"fb{tagp[-1]}",
                            name=f"fb{tagp}")
            out = [fb]
            for p in range(2):
                sp = ipool.tile([128, HW], F16, tag=f"sp{p}", bufs=2,
                                name=f"sp{p}_{tagp}")
                for q in range(4):
                    k = 4 * p + q
                    nc.scalar.activation(sp[:, q * NB:(q + 1) * NB], src_ap,
                                         AF.Abs, bias=bias_ap(3.5 - k),
                                         scale=2.5)
                av = ipool.tile([128, HW], F16, tag=f"av{p}", bufs=2,
                                name=f"av{p}_{tagp}")
                nc.scalar.activation(av[:, :], sp[:, :], AF.Relu,
                                     bias=bias_ap(2.0), scale=-1.0)
                if p == 0:
                    nc.scalar.activation(fb[:, :], src_ap, AF.Silu,
                                         bias=bias_ap(0.0), scale=1.0)
                et = ipool.tile([128, HW], F16, tag=f"et{p}", bufs=1,
                                name=f"et{p}_{tagp}")
                nc.vector.tensor_scalar(et[:, :], av[:, :], 1.0, 0.0,
                                        ALU.subtract, ALU.max)
                a2 = ipool.tile([128, HW], F16, tag=f"p{p}", bufs=1,
                                name=f"a2{p}_{tagp}")
                a3 = ipool.tile([128, HW], F16, tag=f"q{p}", bufs=1,
                                name=f"a3{p}_{tagp}")
                nc.vector.tensor_tensor(a2[:, :], av[:, :], av[:, :],
                                        ALU.mult)
                nc.vector.tensor_tensor(a3[:, :], a2[:, :], av[:, :],
                                        ALU.mult)
                gt = ipool.tile([128, HW], F16, tag=f"p{p}", bufs=1,
                                name=f"gt{p}_{tagp}")
                nc.vector.tensor_scalar(gt[:, :], et[:, :], 2.0, None,
                                        ALU.mult)
                e2 = ipool.tile([128, HW], F16, tag=f"r{p}", bufs=1,
                                name=f"e2{p}_{tagp}")
                nc.vector.tensor_tensor(e2[:, :], gt[:, :], gt[:, :],
                                        ALU.mult)
                e3 = ipool.tile([128, HW], F16, tag=f"p{p}", bufs=1,
                                name=f"e3{p}_{tagp}")
                nc.vector.tensor_tensor(e3[:, :], e2[:, :], et[:, :],
                                        ALU.mult)
                fs = fpool.tile([128, HW], F16, tag=f"fs{p}_{tagp[-1]}",
                                name=f"fs{p}_{tagp}")
                nc.vector.tensor_tensor(fs[:, :], a3[:, :], e3[:, :],
                                        ALU.subtract)
                out.append(fs)
            return out

        def load_w(dram_slice, tagp):
            """Split j=0 block from the rest so j-0 matmuls start early."""
            wb = wpool.tile([128, NB], F16, tag="wb", bufs=3,
                            name=f"wb{tagp}")
            nc.sync.dma_start(out=wb[:, :], in_=dram_slice[:, 0:NB])
            ws = wpool.tile([128, 8 * NB], F16, tag="ws", bufs=3,
                            name=f"ws{tagp}")
            nc.sync.dma_start(out=ws[:, :], in_=dram_slice[:, NB:9 * NB])
            return wb, ws

        def kan_matmuls(pts, whs, fic, ic, n_ic):
            # j-major so early-ready features feed the PE first
            for j in range(9):
                if j == 0:
                    rhs = fic[0][:, :]
                else:
                    rhs = fic[1 + (j - 1) // 4][
                        :, ((j - 1) % 4) * NB:((j - 1) % 4 + 1) * NB]
                for half in range(2):
                    wb, ws = whs[half]
                    for oc4 in range(4):
                        oc = half * 4 + oc4
                        if j == 0:
                            lhsT = wb[:, oc4 * 128:(oc4 + 1) * 128]
                        else:
                            base = (j - 1) * NB + oc4 * 128
                            lhsT = ws[:, base:base + 128]
                        nc.tensor.matmul(
                            pts[oc][:, :], lhsT, rhs,
                            start=(ic == 0 and j == 0),
                            stop=(ic == n_ic - 1 and j == 8))

        # ---- Layer 0: out split in two groups of 8 PSUM banks ----
        for og in range(2):
            pts = [psum.tile([128, NB], F32, tag=f"bank{oc}",
                             name=f"psA{og}_{oc}") for oc in range(8)]
            for ic in range(8):
                if og == 0:
                    xt = ipool.tile([128, NB], F16, tag="xt", bufs=2,
                                    name=f"xt{ic}")
                    nc.sync.dma_start(out=xt[:, :], in_=x_d[ic, :, :])
                    fics[ic] = emit_feats(xt[:, :], f"a{ic}")
                whs = [load_w(w0_d[ic, og, half], f"w0_{og}_{ic}_{half}")
                       for half in range(2)]
                kan_matmuls(pts, whs, fics[ic], ic, 8)
                if og == 1:
                    # recompute feats for layer 1 while og1 matmuls drain
                    fics[ic] = emit_feats(h0ts[ic][:, :], f"b{ic}")
            for oc in range(8):
                h0t = fpool.tile([128, NB], F16, tag=f"h0_{og * 8 + oc}",
                                 name=f"h0t{og * 8 + oc}")
                nc.scalar.activation(h0t[:, :], pts[oc][:, :], AF.Copy,
                                     bias=0.0, scale=1.0)
                h0ts[og * 8 + oc] = h0t

        # head weights: small, load while layer 1 runs
        dwt = opool.tile([128, 8, L], F16, name="dwt")
        for ic in range(8):
            nc.sync.dma_start(out=dwt[:, ic, :], in_=dw_d[ic, :, :])
        dbt = opool.tile([128, 4], F32, name="dbt")
        nc.sync.dma_start(out=dbt[:, :], in_=db_d[:, :])

        # ---- Layer 1: 8 out chunks, 16 contraction chunks ----
        pts = [psum.tile([128, NB], F32, tag=f"bank{oc}", name=f"psB{oc}")
               for oc in range(8)]
        for ic in range(16):
            if ic >= 8:
                fics[ic - 8] = emit_feats(h0ts[ic][:, :], f"c{ic - 8}")
            whs = [load_w(w1_d[ic, half], f"w1_{ic}_{half}")
                   for half in range(2)]
            kan_matmuls(pts, whs, fics[ic % 8], ic, 16)

        # ---- Head: relu(h1) @ dw.T + db ----
        rls = []
        for oc in range(8):
            rl = opool.tile([128, NB], F16, tag=f"rl{oc}", name=f"rl{oc}")
            nc.scalar.activation(rl[:, :], pts[oc][:, :], AF.Relu,
                                 bias=bias_ap(0.0), scale=1.0)
            rls.append(rl)
        for lc in range(4):
            pt = psum.tile([128, NB], F32, tag=f"bank{lc}", name=f"psC{lc}")
            for ic in range(8):
                nc.tensor.matmul(pt[:, :], dwt[:, ic, lc * 128:(lc + 1) * 128],
                                 rls[ic][:, :],
                                 start=(ic == 0), stop=(ic == 7))
            ot = opool.tile([128, NB], F32, tag="ot", bufs=2, name=f"ot{lc}")
            nc.scalar.activation(ot[:, :], pt[:, :], AF.Identity,
                                 bias=dbt[:, lc:lc + 1], scale=1.0)
            nc.sync.dma_start(out=o_d[lc, :, :], in_=ot[:, :])

    nc.compile()
    return nc


def _prep_weights(bw0, sw0, ss0, bw1, sw1, ss1, dw, db):
    # KAN layer weights: slot j=0 -> base weight, j=1+k -> sw*ss/6
    W0 = np.empty((D_IN, 9, H0), np.float32)
    W0[:, 0, :] = bw0.T
    W0[:, 1:, :] = (sw0 * (ss0[:, :, None] / 6.0)).transpose(1, 2, 0)
    w0 = np.ascontiguousarray(
        W0.reshape(8, 128, 9, 2, 2, 4, 128)
          .transpose(0, 3, 4, 1, 2, 5, 6)
          .reshape(8, 2, 2, 128, 9 * NB)).astype(np.float16)
    W1 = np.empty((H0, 9, H1), np.float32)
    W1[:, 0, :] = bw1.T
    W1[:, 1:, :] = (sw1 * (ss1[:, :, None] / 6.0)).transpose(1, 2, 0)
    w1 = np.ascontiguousarray(
        W1.reshape(16, 128, 9, 2, 4, 128)
          .transpose(0, 3, 1, 2, 4, 5)
          .reshape(16, 2, 128, 9 * NB)).astype(np.float16)
    dwt = np.ascontiguousarray(dw.T.reshape(8, 128, L)).astype(np.float16)
    dbt = np.ascontiguousarray(db.reshape(4, 128).T.astype(np.float32))
    return w0, w1, dwt, dbt


def kernel(x, bw0, sw0, ss0, bw1, sw1, ss1, dw, db):
    if "nc" not in _cache:
        _cache["nc"] = _build_full()
    nc = _cache["nc"]
    w0, w1, dwt, dbt = _prep_weights(
        np.asarray(bw0, np.float32), np.asarray(sw0, np.float32),
        np.asarray(ss0, np.float32), np.asarray(bw1, np.float32),
        np.asarray(sw1, np.float32), np.asarray(ss1, np.float32),
        np.asarray(dw, np.float32), np.asarray(db, np.float32))
    xT = np.ascontiguousarray(np.asarray(x, np.float32).T).astype(np.float16)
    in_maps = []
    for c in range(NCORES):
        xc = np.ascontiguousarray(
            xT[:, c * BC:(c + 1) * BC].reshape(8, 128, BC))
        in_maps.append({"x_d": xc, "w0_d": w0, "w1_d": w1,
                        "dw_d": dwt, "db_d": dbt})
    _cache["in_maps"] = in_maps
    res = run_bass_kernel_spmd(nc, in_maps, list(range(NCORES)))
    out = np.empty((B, L), np.float32)
    for c in range(NCORES):
        oc = res.results[c]["o_d"]          # (4, 128, BC)
        out[c * BC:(c + 1) * BC, :] = oc.reshape(L, BC).T
    return out
